# revision 18
# baseline (speedup 1.0000x reference)
"""Multi-head causal attention (dense transformer block) on 8 TRN2 NeuronCores.

Problem: B=2, S=2048, D_MODEL=768, H=12 heads, D_HEAD=64, fp32 I/O.

Sharding: 24 (batch, head) units over 8 cores -> 3 heads x 1 batch per core.
Cores 0-3 handle batch 0 (heads 0-2, 3-5, 6-8, 9-11), cores 4-7 batch 1.
Each core computes its heads' contribution to out[b] = sum_h z_h @ W_O[h];
the host sums the 4 partials per batch and adds b_O (an all-reduce done on
the host since the kernel contract is full-in/full-out).

Per-core dataflow (all matmuls bf16 -> fp32 PSUM):
  - x[b]^T staged in SBUF as 6 chunks [128, 2048].
  - QT/KT in "head-transposed" layout [64, S] packed 2-per-tile:
      QT01 = [Q_h0*s | Q_h1*s]^T x  (scale 1/sqrt(64) folded into W_Q/b_Q)
      KT01 = [K_h0 | K_h1]^T x, QKT2 = [Q_h2*s | K_h2]^T x
  - scores TRANSPOSED: sT[k, q] = KT^T-slice x QT (K=64 contraction), so
    softmax normalization is a per-q (free dim) reciprocal and no PE
    transposes of the probability matrix are ever needed.
  - exp via ScalarE (one instr per [128, 1024] j-pair group), causal mask via
    memset + one triangular-mask multiply on the diagonal 128x128 tile.
  - zT[h] = sum_j V[j]^T-slice x PT[j]  (unnormalized), denominators via a
    ones-vector matmul (M=1) accumulated alongside, applied as
    zT * broadcast(1/sums) when copying PSUM->SBUF.
  - out[q, :] = zT01^T-slice @ WO01 + zT2^T-slice @ WO2, DMA'd PSUM->DRAM.
"""

import numpy as np
import ml_dtypes
from contextlib import ExitStack

import concourse.bass as bass
import concourse.mybir as mybir
import concourse.tile as tile
from concourse import bacc
from concourse.bass_utils import run_bass_kernel_spmd

BF16 = mybir.dt.bfloat16
F32 = mybir.dt.float32
AF = mybir.ActivationFunctionType
NPBF16 = ml_dtypes.bfloat16

B, S, D, H, DH = 2, 2048, 768, 12, 64
N_CORES = 8
DCH = D // 128          # 6 d_model chunks
NKT = S // 128          # 16 k tiles
QB = 512                # q block width
NQB = S // QB           # 4 q blocks
G = 2                   # k-tiles per exp group

TRACE_ENABLED = False
LAST_EXEC_NS = None
_BUILT = None


def build_nc():
    nc = bacc.Bacc("TRN2", target_bir_lowering=False, debug=False)

    xT_d = nc.dram_tensor("xT", [D, S], BF16, kind="ExternalInput")
    wq01_d = nc.dram_tensor("wq01", [D, 128], BF16, kind="ExternalInput")
    wk01_d = nc.dram_tensor("wk01", [D, 128], BF16, kind="ExternalInput")
    wqk2_d = nc.dram_tensor("wqk2", [D, 128], BF16, kind="ExternalInput")
    wv_d = nc.dram_tensor("wv", [D, 192], BF16, kind="ExternalInput")
    wo01_d = nc.dram_tensor("wo01", [128, D], BF16, kind="ExternalInput")
    wo2_d = nc.dram_tensor("wo2", [64, D], BF16, kind="ExternalInput")
    bq01_d = nc.dram_tensor("bq01", [128, 1], F32, kind="ExternalInput")
    bk01_d = nc.dram_tensor("bk01", [128, 1], F32, kind="ExternalInput")
    bqk2_d = nc.dram_tensor("bqk2", [128, 1], F32, kind="ExternalInput")
    bv_d = nc.dram_tensor("bv", [128, 192], F32, kind="ExternalInput")
    out_d = nc.dram_tensor("out_p", [S, D], F32, kind="ExternalOutput")

    tri_np = np.triu(np.ones((128, 128), np.float32)).astype(NPBF16)
    tri_d = nc.inline_tensor(tri_np, "tri")
    ones_d = nc.inline_tensor(np.ones((128, 1), NPBF16), "ones_w")

    with tile.TileContext(nc) as tc, ExitStack() as ctx:
        persist = ctx.enter_context(tc.tile_pool(name="persist", bufs=1))

        # ---- stage inputs in SBUF ----
        xt = []
        for d in range(DCH):
            t = persist.tile([128, S], BF16, tag=f"xt{d}")
            nc.sync.dma_start(t[:], xT_d[d * 128:(d + 1) * 128, :])
            xt.append(t)

        def load_w(dram, cols, tag):
            t = persist.tile([128, DCH * cols], BF16, tag=tag)
            for d in range(DCH):
                nc.sync.dma_start(t[:, d * cols:(d + 1) * cols],
                                  dram[d * 128:(d + 1) * 128, :])
            return t

        wq01 = load_w(wq01_d, 128, "wq01")
        wk01 = load_w(wk01_d, 128, "wk01")
        wqk2 = load_w(wqk2_d, 128, "wqk2")
        wv = load_w(wv_d, 192, "wv")

        wo01 = persist.tile([128, D], BF16, tag="wo01")
        nc.sync.dma_start(wo01[:], wo01_d[:, :])
        wo2 = persist.tile([64, D], BF16, tag="wo2")
        nc.sync.dma_start(wo2[:], wo2_d[:, :])

        def load_small(dram, shape, dt, tag):
            t = persist.tile(shape, dt, tag=tag)
            nc.sync.dma_start(t[:], dram[:, :])
            return t

        bq01 = load_small(bq01_d, [128, 1], F32, "bq01")
        bk01 = load_small(bk01_d, [128, 1], F32, "bk01")
        bqk2 = load_small(bqk2_d, [128, 1], F32, "bqk2")
        bv = load_small(bv_d, [128, 192], F32, "bv")
        tri = load_small(tri_d, [128, 128], BF16, "tri")
        ones_w = load_small(ones_d, [128, 1], BF16, "ones_w")

        # ---- persistent intermediates ----
        QT01 = persist.tile([128, S], BF16, tag="QT01")
        KT01 = persist.tile([128, S], BF16, tag="KT01")
        QKT2 = persist.tile([128, S], BF16, tag="QKT2")
        KT2lo = persist.tile([64, S], BF16, tag="KT2lo")
        v_sb = persist.tile([128, NKT * 192], BF16, tag="v_sb")
        zT01 = persist.tile([128, S], BF16, tag="zT01")
        zT2 = persist.tile([64, S], BF16, tag="zT2")

        # ---- QKV projections ----
        with tc.tile_pool(name="proj_ps", bufs=1, space="PSUM") as proj_pool, \
             tc.tile_pool(name="v_ps", bufs=2, space="PSUM") as v_pool:
            for w_s, bias_s, out_s in ((wq01, bq01, QT01),
                                       (wk01, bk01, KT01),
                                       (wqk2, bqk2, QKT2)):
                ps = proj_pool.tile([128, S], F32, tag="chain")
                for d in range(DCH):
                    lhsT = w_s[:, d * 128:(d + 1) * 128]
                    for n in range(S // 512):
                        nc.tensor.matmul(ps[:, n * 512:(n + 1) * 512], lhsT,
                                         xt[d][:, n * 512:(n + 1) * 512],
                                         start=(d == 0), stop=(d == DCH - 1))
                for n in range(S // 512):
                    nsl = slice(n * 512, (n + 1) * 512)
                    nc.vector.tensor_scalar_add(out_s[:, nsl], ps[:, nsl],
                                                bias_s[:])

            for s_t in range(NKT):
                ps = v_pool.tile([128, 192], F32, tag="vps")
                for d in range(DCH):
                    nc.tensor.matmul(ps[:], xt[d][:, s_t * 128:(s_t + 1) * 128],
                                     wv[:, d * 192:(d + 1) * 192],
                                     start=(d == 0), stop=(d == DCH - 1))
                nc.vector.tensor_add(v_sb[:, s_t * 192:(s_t + 1) * 192], ps[:], bv[:])

        # head-2 K^T copy to partitions 0-63 (scores h2 needs lhsT and rhs on
        # the same partition range)
        nc.sync.dma_start(KT2lo[:], QKT2[64:128, :])

        # ---- attention ----
        with tc.tile_pool(name="sT_ps", bufs=2, space="PSUM") as sT_pool, \
             tc.tile_pool(name="zT_ps", bufs=2, space="PSUM") as zT_pool, \
             tc.tile_pool(name="sums_ps", bufs=2, space="PSUM") as sums_pool, \
             tc.tile_pool(name="pt_sb", bufs=4) as pt_pool, \
             tc.tile_pool(name="rb_sb", bufs=2) as rb_pool, \
             tc.tile_pool(name="recip_dr", bufs=3, space="DRAM") as rdr_pool, \
             tc.tile_pool(name="recip_sb", bufs=3) as recip_pool:

            def bcast_recip(sums_psum, rb_slice, nchan):
                """1/sums broadcast across partitions, via a DRAM bounce
                (SBUF-source partition-broadcast DMA is rejected)."""
                rc = recip_pool.tile([1, 512], F32, tag="recip")
                nc.vector.reciprocal(rc[:], sums_psum[:])
                dr = rdr_pool.tile([1, 512], F32, tag="rdr")
                nc.sync.dma_start(dr[:], rc[:])
                nc.sync.dma_start(rb_slice,
                                  dr[:].broadcast_to([nchan, 512]))

            for qi in range(NQB):
                q0 = qi * QB
                J = 4 * qi + 4
                qsl = slice(q0, q0 + QB)

                def head_loop(score_lhs, score_rhs, po, pv_voff, zt_out, sum_t):
                    """Emit scores+exp+mask+PV+sums for one head's (qi) row.

                    po: partition offset (0 or 64) of this head's K^T/Q^T rows
                    inside score_lhs/score_rhs — also selects the PE row strip.
                    """
                    for g in range(J // G):
                        st = sT_pool.tile([128, G * 512], F32, tag="sT")
                        for jj in range(G):
                            j = g * G + jj
                            nc.tensor.matmul(
                                st[:, jj * 512:(jj + 1) * 512],
                                score_lhs[po:po + 64, j * 128:(j + 1) * 128],
                                score_rhs[po:po + 64, qsl],
                                start=True, stop=True)
                        pt = pt_pool.tile([128, G * 512], BF16, tag="pt")
                        j0 = g * G
                        r0 = j0 - 4 * qi
                        s0 = r0 * 128 if r0 >= 0 else 0
                        nc.scalar.activation(pt[:, s0:G * 512], st[:, s0:G * 512],
                                             AF.Exp)
                        for jj in range(G):
                            j = g * G + jj
                            r = j - 4 * qi
                            off = jj * 512
                            if r >= 0:
                                if r > 0:
                                    nc.vector.memset(pt[:, off:off + r * 128], 0.0)
                                dsl = slice(off + r * 128, off + (r + 1) * 128)
                                nc.vector.tensor_mul(pt[:, dsl], pt[:, dsl], tri[:])
                        for jj in range(G):
                            j = g * G + jj
                            nc.tensor.matmul(
                                zt_out[:],
                                v_sb[:, j * 192 + pv_voff:j * 192 + pv_voff + 64],
                                pt[:, jj * 512:(jj + 1) * 512],
                                start=(j == 0), stop=(j == J - 1))
                            nc.tensor.matmul(
                                sum_t[:], ones_w[:],
                                pt[:, jj * 512:(jj + 1) * 512],
                                start=(j == 0), stop=(j == J - 1))

                # heads 0 and 1 (paired on row strips of the PE array)
                zt_pair = zT_pool.tile([128, 512], F32, tag="zT")
                sums0 = sums_pool.tile([1, 512], F32, tag="sums")
                sums1 = sums_pool.tile([1, 512], F32, tag="sums")
                head_loop(KT01, QT01, 0, 0, zt_pair[0:64, :], sums0)
                head_loop(KT01, QT01, 64, 64, zt_pair[64:128, :], sums1)

                rb = rb_pool.tile([128, 512], F32, tag="rb")
                bcast_recip(sums0, rb[0:64, :], 64)
                bcast_recip(sums1, rb[64:128, :], 64)
                nc.vector.tensor_mul(zT01[:, qsl], zt_pair[:], rb[:])

                # head 2
                zt2 = zT_pool.tile([64, 512], F32, tag="zT")
                sums2 = sums_pool.tile([1, 512], F32, tag="sums")
                head_loop(KT2lo, QKT2, 0, 128, zt2[:], sums2)

                rb2 = rb_pool.tile([128, 512], F32, tag="rb")
                bcast_recip(sums2, rb2[0:64, :], 64)
                nc.vector.tensor_mul(zT2[:, qsl], zt2[:], rb2[0:64, :])

        # ---- output projection ----
        with tc.tile_pool(name="op_ps", bufs=2, space="PSUM") as op_pool, \
             tc.tile_pool(name="out_sb", bufs=3) as out_pool:
            for t in range(NKT):
                ps = op_pool.tile([128, D], F32, tag="op")
                tsl = slice(t * 128, (t + 1) * 128)
                for n0, nw in ((0, 512), (512, 256)):
                    nc.tensor.matmul(ps[:, n0:n0 + nw], zT01[:, tsl],
                                     wo01[:, n0:n0 + nw], start=True, stop=False)
                for n0, nw in ((0, 512), (512, 256)):
                    nc.tensor.matmul(ps[:, n0:n0 + nw], zT2[:, tsl],
                                     wo2[:, n0:n0 + nw], start=False, stop=True)
                ob = out_pool.tile([128, D], F32, tag="ob")
                nc.vector.tensor_copy(ob[:], ps[:])
                nc.sync.dma_start(out_d[tsl, :], ob[:])

    nc.compile()
    return nc


def _get_nc():
    global _BUILT
    if _BUILT is None:
        _BUILT = build_nc()
    return _BUILT


def make_in_maps(inputs):
    x = np.asarray(inputs["normalized_resid_pre"], dtype=np.float32)
    W_Q = np.asarray(inputs["W_Q"], dtype=np.float32)
    W_K = np.asarray(inputs["W_K"], dtype=np.float32)
    W_V = np.asarray(inputs["W_V"], dtype=np.float32)
    W_O = np.asarray(inputs["W_O"], dtype=np.float32)
    b_Q = np.asarray(inputs["b_Q"], dtype=np.float32)
    b_K = np.asarray(inputs["b_K"], dtype=np.float32)
    b_V = np.asarray(inputs["b_V"], dtype=np.float32)
    sc = 1.0 / np.sqrt(np.float32(DH))

    in_maps = []
    for c in range(N_CORES):
        b = c // 4
        h = (c % 4) * 3
        hs = [h, h + 1, h + 2]
        m = {
            "xT": np.ascontiguousarray(x[b].T).astype(NPBF16),
            "wq01": np.concatenate([W_Q[hs[0]] * sc, W_Q[hs[1]] * sc],
                                   axis=1).astype(NPBF16),
            "wk01": np.concatenate([W_K[hs[0]], W_K[hs[1]]], axis=1).astype(NPBF16),
            "wqk2": np.concatenate([W_Q[hs[2]] * sc, W_K[hs[2]]],
                                   axis=1).astype(NPBF16),
            "wv": np.concatenate([W_V[hh] for hh in hs], axis=1).astype(NPBF16),
            "wo01": np.concatenate([W_O[hs[0]], W_O[hs[1]]], axis=0).astype(NPBF16),
            "wo2": W_O[hs[2]].astype(NPBF16),
            "bq01": (np.concatenate([b_Q[hs[0]], b_Q[hs[1]]]) * sc)[:, None]
                    .astype(np.float32),
            "bk01": np.concatenate([b_K[hs[0]], b_K[hs[1]]])[:, None]
                    .astype(np.float32),
            "bqk2": np.concatenate([b_Q[hs[2]] * sc, b_K[hs[2]]])[:, None]
                    .astype(np.float32),
            "bv": np.ascontiguousarray(
                np.broadcast_to(np.concatenate([b_V[hh] for hh in hs]),
                                (128, 192))).astype(np.float32),
        }
        in_maps.append(m)
    return in_maps


def kernel(**inputs):
    global LAST_EXEC_NS
    nc = _get_nc()
    in_maps = make_in_maps(inputs)
    b_O = np.asarray(inputs["b_O"], dtype=np.float32)

    res = run_bass_kernel_spmd(nc, in_maps, core_ids=list(range(N_CORES)),
                               trace=TRACE_ENABLED)
    LAST_EXEC_NS = res.exec_time_ns
    parts = [r["out_p"] for r in res.results]
    out0 = parts[0] + parts[1] + parts[2] + parts[3]
    out1 = parts[4] + parts[5] + parts[6] + parts[7]
    out = np.stack([out0, out1]) + b_O
    return out.astype(np.float32)


# revision 20
# speedup vs baseline: 1.0120x; 1.0120x over previous
"""Multi-head causal attention (dense transformer block) on 8 TRN2 NeuronCores.

Problem: B=2, S=2048, D_MODEL=768, H=12 heads, D_HEAD=64, fp32 I/O.

Sharding: 24 (batch, head) units over 8 cores -> 3 heads x 1 batch per core.
Cores 0-3 handle batch 0 (heads 0-2, 3-5, 6-8, 9-11), cores 4-7 batch 1.
Each core computes its heads' contribution to out[b] = sum_h z_h @ W_O[h];
the host sums the 4 partials per batch and adds b_O (an all-reduce done on
the host since the kernel contract is full-in/full-out).

Per-core dataflow (all matmuls bf16 -> fp32 PSUM):
  - x[b]^T staged in SBUF as 6 chunks [128, 2048].
  - QT/KT in "head-transposed" layout [64, S] packed 2-per-tile:
      QT01 = [Q_h0*s | Q_h1*s]^T x  (scale 1/sqrt(64) folded into W_Q/b_Q)
      KT01 = [K_h0 | K_h1]^T x, QKT2 = [Q_h2*s | K_h2]^T x
  - scores TRANSPOSED: sT[k, q] = KT^T-slice x QT (K=64 contraction), so
    softmax normalization is a per-q (free dim) reciprocal and no PE
    transposes of the probability matrix are ever needed.
  - exp via ScalarE (one instr per [128, 1024] j-pair group), causal mask via
    memset + one triangular-mask multiply on the diagonal 128x128 tile.
  - zT[h] = sum_j V[j]^T-slice x PT[j]  (unnormalized), denominators via a
    ones-vector matmul (M=1) accumulated alongside, applied as
    zT * broadcast(1/sums) when copying PSUM->SBUF.
  - out[q, :] = zT01^T-slice @ WO01 + zT2^T-slice @ WO2, DMA'd PSUM->DRAM.
"""

import numpy as np
import ml_dtypes
from contextlib import ExitStack

import concourse.bass as bass
import concourse.mybir as mybir
import concourse.tile as tile
from concourse import bacc
from concourse.bass_utils import run_bass_kernel_spmd

BF16 = mybir.dt.bfloat16
F32 = mybir.dt.float32
AF = mybir.ActivationFunctionType
NPBF16 = ml_dtypes.bfloat16

B, S, D, H, DH = 2, 2048, 768, 12, 64
N_CORES = 8
DCH = D // 128          # 6 d_model chunks
NKT = S // 128          # 16 k tiles
QB = 512                # q block width
NQB = S // QB           # 4 q blocks
G = 2                   # k-tiles per exp group

TRACE_ENABLED = False
LAST_EXEC_NS = None
LAST_RESULT = None
_BUILT = None


def build_nc():
    nc = bacc.Bacc("TRN2", target_bir_lowering=False, debug=False)

    xT_d = nc.dram_tensor("xT", [D, S], BF16, kind="ExternalInput")
    wq01_d = nc.dram_tensor("wq01", [D, 128], BF16, kind="ExternalInput")
    wk01_d = nc.dram_tensor("wk01", [D, 128], BF16, kind="ExternalInput")
    wqk2_d = nc.dram_tensor("wqk2", [D, 128], BF16, kind="ExternalInput")
    wv_d = nc.dram_tensor("wv", [D, 192], BF16, kind="ExternalInput")
    wo01_d = nc.dram_tensor("wo01", [128, D], BF16, kind="ExternalInput")
    wo2_d = nc.dram_tensor("wo2", [64, D], BF16, kind="ExternalInput")
    bq01_d = nc.dram_tensor("bq01", [128, 1], F32, kind="ExternalInput")
    bk01_d = nc.dram_tensor("bk01", [128, 1], F32, kind="ExternalInput")
    bqk2_d = nc.dram_tensor("bqk2", [128, 1], F32, kind="ExternalInput")
    bv_d = nc.dram_tensor("bv", [128, 192], F32, kind="ExternalInput")
    out_d = nc.dram_tensor("out_p", [S, D], F32, kind="ExternalOutput")

    tri_np = np.triu(np.ones((128, 128), np.float32)).astype(NPBF16)
    tri_d = nc.inline_tensor(tri_np, "tri")
    ones_d = nc.inline_tensor(np.ones((128, 1), NPBF16), "ones_w")

    with tile.TileContext(nc) as tc, ExitStack() as ctx:
        persist = ctx.enter_context(tc.tile_pool(name="persist", bufs=1))

        # ---- stage inputs in SBUF ----
        xt = []
        for d in range(DCH):
            t = persist.tile([128, S], BF16, tag=f"xt{d}")
            nc.sync.dma_start(t[:], xT_d[d * 128:(d + 1) * 128, :])
            xt.append(t)

        def load_w(dram, cols, tag):
            t = persist.tile([128, DCH * cols], BF16, tag=tag)
            for d in range(DCH):
                nc.sync.dma_start(t[:, d * cols:(d + 1) * cols],
                                  dram[d * 128:(d + 1) * 128, :])
            return t

        wq01 = load_w(wq01_d, 128, "wq01")
        wk01 = load_w(wk01_d, 128, "wk01")
        wqk2 = load_w(wqk2_d, 128, "wqk2")
        wv = load_w(wv_d, 192, "wv")

        wo01 = persist.tile([128, D], BF16, tag="wo01")
        nc.sync.dma_start(wo01[:], wo01_d[:, :])
        wo2 = persist.tile([64, D], BF16, tag="wo2")
        nc.sync.dma_start(wo2[:], wo2_d[:, :])

        def load_small(dram, shape, dt, tag):
            t = persist.tile(shape, dt, tag=tag)
            nc.sync.dma_start(t[:], dram[:, :])
            return t

        bq01 = load_small(bq01_d, [128, 1], F32, "bq01")
        bk01 = load_small(bk01_d, [128, 1], F32, "bk01")
        bqk2 = load_small(bqk2_d, [128, 1], F32, "bqk2")
        bv = load_small(bv_d, [128, 192], F32, "bv")
        tri = load_small(tri_d, [128, 128], BF16, "tri")
        ones_w = load_small(ones_d, [128, 1], BF16, "ones_w")

        # ---- persistent intermediates ----
        QT01 = persist.tile([128, S], BF16, tag="QT01")
        KT01 = persist.tile([128, S], BF16, tag="KT01")
        QKT2 = persist.tile([128, S], BF16, tag="QKT2")
        KT2lo = persist.tile([64, S], BF16, tag="KT2lo")
        v_sb = persist.tile([128, NKT * 192], BF16, tag="v_sb")
        zT01 = persist.tile([128, S], BF16, tag="zT01")
        zT2 = persist.tile([64, S], BF16, tag="zT2")

        # ---- QKV projections ----
        with tc.tile_pool(name="proj_ps", bufs=1, space="PSUM") as proj_pool, \
             tc.tile_pool(name="v_ps", bufs=2, space="PSUM") as v_pool:
            for w_s, bias_s, out_s in ((wq01, bq01, QT01),
                                       (wk01, bk01, KT01),
                                       (wqk2, bqk2, QKT2)):
                ps = proj_pool.tile([128, S], F32, tag="chain")
                for d in range(DCH):
                    lhsT = w_s[:, d * 128:(d + 1) * 128]
                    for n in range(S // 512):
                        nc.tensor.matmul(ps[:, n * 512:(n + 1) * 512], lhsT,
                                         xt[d][:, n * 512:(n + 1) * 512],
                                         start=(d == 0), stop=(d == DCH - 1))
                for n in range(S // 512):
                    nsl = slice(n * 512, (n + 1) * 512)
                    nc.vector.tensor_scalar_add(out_s[:, nsl], ps[:, nsl],
                                                bias_s[:])

            for s_t in range(NKT):
                ps = v_pool.tile([128, 192], F32, tag="vps")
                for d in range(DCH):
                    nc.tensor.matmul(ps[:], xt[d][:, s_t * 128:(s_t + 1) * 128],
                                     wv[:, d * 192:(d + 1) * 192],
                                     start=(d == 0), stop=(d == DCH - 1))
                nc.vector.tensor_add(v_sb[:, s_t * 192:(s_t + 1) * 192], ps[:], bv[:])

        # head-2 K^T copy to partitions 0-63 (scores h2 needs lhsT and rhs on
        # the same partition range)
        nc.sync.dma_start(KT2lo[:], QKT2[64:128, :])

        # ---- attention ----
        with tc.tile_pool(name="sT_ps", bufs=2, space="PSUM") as sT_pool, \
             tc.tile_pool(name="zT_ps", bufs=2, space="PSUM") as zT_pool, \
             tc.tile_pool(name="sums_ps", bufs=2, space="PSUM") as sums_pool, \
             tc.tile_pool(name="pt_sb", bufs=4) as pt_pool, \
             tc.tile_pool(name="rb_sb", bufs=2) as rb_pool, \
             tc.tile_pool(name="recip_dr", bufs=3, space="DRAM") as rdr_pool, \
             tc.tile_pool(name="recip_sb", bufs=3) as recip_pool:

            def bcast_recip(sums_psum, rb_slice, nchan):
                """1/sums broadcast across partitions, via a DRAM bounce
                (SBUF-source partition-broadcast DMA is rejected)."""
                rc = recip_pool.tile([1, 512], F32, tag="recip")
                nc.vector.reciprocal(rc[:], sums_psum[:])
                dr = rdr_pool.tile([1, 512], F32, tag="rdr")
                nc.sync.dma_start(dr[:], rc[:])
                nc.sync.dma_start(rb_slice,
                                  dr[:].broadcast_to([nchan, 512]))

            for qi in range(NQB):
                q0 = qi * QB
                J = 4 * qi + 4
                qsl = slice(q0, q0 + QB)

                def head_loop(score_lhs, score_rhs, po, pv_voff, zt_out, sum_t):
                    """Emit scores+exp+mask+PV+sums for one head's (qi) row.

                    po: partition offset (0 or 64) of this head's K^T/Q^T rows
                    inside score_lhs/score_rhs — also selects the PE row strip.
                    """
                    for g in range(J // G):
                        st = sT_pool.tile([128, G * 512], F32, tag="sT")
                        for jj in range(G):
                            j = g * G + jj
                            nc.tensor.matmul(
                                st[:, jj * 512:(jj + 1) * 512],
                                score_lhs[po:po + 64, j * 128:(j + 1) * 128],
                                score_rhs[po:po + 64, qsl],
                                start=True, stop=True)
                        pt = pt_pool.tile([128, G * 512], BF16, tag="pt")
                        j0 = g * G
                        r0 = j0 - 4 * qi
                        s0 = r0 * 128 if r0 >= 0 else 0
                        nc.scalar.activation(pt[:, s0:G * 512], st[:, s0:G * 512],
                                             AF.Exp)
                        for jj in range(G):
                            j = g * G + jj
                            r = j - 4 * qi
                            off = jj * 512
                            if r >= 0:
                                if r > 0:
                                    nc.vector.memset(pt[:, off:off + r * 128], 0.0)
                                dsl = slice(off + r * 128, off + (r + 1) * 128)
                                nc.vector.tensor_mul(pt[:, dsl], pt[:, dsl], tri[:])
                        for jj in range(G):
                            j = g * G + jj
                            nc.tensor.matmul(
                                zt_out[:],
                                v_sb[:, j * 192 + pv_voff:j * 192 + pv_voff + 64],
                                pt[:, jj * 512:(jj + 1) * 512],
                                start=(j == 0), stop=(j == J - 1))
                            nc.tensor.matmul(
                                sum_t[:], ones_w[:],
                                pt[:, jj * 512:(jj + 1) * 512],
                                start=(j == 0), stop=(j == J - 1))

                # heads 0 and 1 (paired on row strips of the PE array)
                zt_pair = zT_pool.tile([128, 512], F32, tag="zT")
                sums0 = sums_pool.tile([1, 512], F32, tag="sums")
                sums1 = sums_pool.tile([1, 512], F32, tag="sums")
                head_loop(KT01, QT01, 0, 0, zt_pair[0:64, :], sums0)
                head_loop(KT01, QT01, 64, 64, zt_pair[64:128, :], sums1)

                rb = rb_pool.tile([128, 512], F32, tag="rb")
                bcast_recip(sums0, rb[0:64, :], 64)
                bcast_recip(sums1, rb[64:128, :], 64)
                nc.vector.tensor_mul(zT01[:, qsl], zt_pair[:], rb[:])

                # head 2
                zt2 = zT_pool.tile([64, 512], F32, tag="zT")
                sums2 = sums_pool.tile([1, 512], F32, tag="sums")
                head_loop(KT2lo, QKT2, 0, 128, zt2[:], sums2)

                rb2 = rb_pool.tile([128, 512], F32, tag="rb")
                bcast_recip(sums2, rb2[0:64, :], 64)
                nc.vector.tensor_mul(zT2[:, qsl], zt2[:], rb2[0:64, :])

        # ---- output projection ----
        with tc.tile_pool(name="op_ps", bufs=2, space="PSUM") as op_pool, \
             tc.tile_pool(name="out_sb", bufs=3) as out_pool:
            for t in range(NKT):
                ps = op_pool.tile([128, D], F32, tag="op")
                tsl = slice(t * 128, (t + 1) * 128)
                for n0, nw in ((0, 512), (512, 256)):
                    nc.tensor.matmul(ps[:, n0:n0 + nw], zT01[:, tsl],
                                     wo01[:, n0:n0 + nw], start=True, stop=False)
                for n0, nw in ((0, 512), (512, 256)):
                    nc.tensor.matmul(ps[:, n0:n0 + nw], zT2[:, tsl],
                                     wo2[:, n0:n0 + nw], start=False, stop=True)
                ob = out_pool.tile([128, D], F32, tag="ob")
                nc.vector.tensor_copy(ob[:], ps[:])
                nc.sync.dma_start(out_d[tsl, :], ob[:])

    nc.compile()
    return nc


def _get_nc():
    global _BUILT
    if _BUILT is None:
        _BUILT = build_nc()
    return _BUILT


def make_in_maps(inputs):
    x = np.asarray(inputs["normalized_resid_pre"], dtype=np.float32)
    W_Q = np.asarray(inputs["W_Q"], dtype=np.float32)
    W_K = np.asarray(inputs["W_K"], dtype=np.float32)
    W_V = np.asarray(inputs["W_V"], dtype=np.float32)
    W_O = np.asarray(inputs["W_O"], dtype=np.float32)
    b_Q = np.asarray(inputs["b_Q"], dtype=np.float32)
    b_K = np.asarray(inputs["b_K"], dtype=np.float32)
    b_V = np.asarray(inputs["b_V"], dtype=np.float32)
    sc = 1.0 / np.sqrt(np.float32(DH))

    in_maps = []
    for c in range(N_CORES):
        b = c // 4
        h = (c % 4) * 3
        hs = [h, h + 1, h + 2]
        m = {
            "xT": np.ascontiguousarray(x[b].T).astype(NPBF16),
            "wq01": np.concatenate([W_Q[hs[0]] * sc, W_Q[hs[1]] * sc],
                                   axis=1).astype(NPBF16),
            "wk01": np.concatenate([W_K[hs[0]], W_K[hs[1]]], axis=1).astype(NPBF16),
            "wqk2": np.concatenate([W_Q[hs[2]] * sc, W_K[hs[2]]],
                                   axis=1).astype(NPBF16),
            "wv": np.concatenate([W_V[hh] for hh in hs], axis=1).astype(NPBF16),
            "wo01": np.concatenate([W_O[hs[0]], W_O[hs[1]]], axis=0).astype(NPBF16),
            "wo2": W_O[hs[2]].astype(NPBF16),
            "bq01": (np.concatenate([b_Q[hs[0]], b_Q[hs[1]]]) * sc)[:, None]
                    .astype(np.float32),
            "bk01": np.concatenate([b_K[hs[0]], b_K[hs[1]]])[:, None]
                    .astype(np.float32),
            "bqk2": np.concatenate([b_Q[hs[2]] * sc, b_K[hs[2]]])[:, None]
                    .astype(np.float32),
            "bv": np.ascontiguousarray(
                np.broadcast_to(np.concatenate([b_V[hh] for hh in hs]),
                                (128, 192))).astype(np.float32),
        }
        in_maps.append(m)
    return in_maps


def kernel(**inputs):
    global LAST_EXEC_NS, LAST_RESULT
    nc = _get_nc()
    in_maps = make_in_maps(inputs)
    b_O = np.asarray(inputs["b_O"], dtype=np.float32)

    res = run_bass_kernel_spmd(nc, in_maps, core_ids=list(range(N_CORES)),
                               trace=TRACE_ENABLED)
    LAST_EXEC_NS = res.exec_time_ns
    LAST_RESULT = res
    parts = [r["out_p"] for r in res.results]
    out0 = parts[0] + parts[1] + parts[2] + parts[3]
    out1 = parts[4] + parts[5] + parts[6] + parts[7]
    out = np.stack([out0, out1]) + b_O
    return out.astype(np.float32)


# revision 34
# speedup vs baseline: 1.1694x; 1.1556x over previous
"""Multi-head causal attention (dense transformer block) on 8 TRN2 NeuronCores.

Problem: B=2, S=2048, D_MODEL=768, H=12 heads, D_HEAD=64, fp32 I/O.

Sharding: 24 (batch, head) units over 8 cores -> 3 heads x 1 batch per core.
Cores 0-3 handle batch 0 (heads 0-2, 3-5, 6-8, 9-11), cores 4-7 batch 1.
Each core computes its heads' contribution to out[b] = sum_h z_h @ W_O[h];
the host sums the 4 partials per batch and adds b_O (an all-reduce done on
the host since the kernel contract is full-in/full-out).

Per-core dataflow (all matmuls bf16 -> fp32 PSUM):
  - x[b]^T staged in SBUF as 6 chunks [128, 2048].
  - QT/KT in "head-transposed" layout [64, S] packed 2-per-tile:
      QT01 = [Q_h0*s | Q_h1*s]^T x  (scale 1/sqrt(64) folded into W_Q/b_Q)
      KT01 = [K_h0 | K_h1]^T x, QKT2 = [Q_h2*s | K_h2]^T x
  - scores TRANSPOSED: sT[k, q] = KT^T-slice x QT (K=64 contraction), so
    softmax normalization is a per-q (free dim) reciprocal and no PE
    transposes of the probability matrix are ever needed.
  - exp via ScalarE (one instr per [128, 1024] j-pair group), causal mask via
    memset + one triangular-mask multiply on the diagonal 128x128 tile.
  - zT[h] = sum_j V[j]^T-slice x PT[j]  (unnormalized), denominators via a
    ones-vector matmul (M=1) accumulated alongside, applied as
    zT * broadcast(1/sums) when copying PSUM->SBUF.
  - out[q, :] = zT01^T-slice @ WO01 + zT2^T-slice @ WO2, DMA'd PSUM->DRAM.
"""

import numpy as np
import ml_dtypes
from contextlib import ExitStack

import concourse.bass as bass
import concourse.mybir as mybir
import concourse.tile as tile
from concourse import bacc
from concourse.bass_utils import run_bass_kernel_spmd

BF16 = mybir.dt.bfloat16
F32 = mybir.dt.float32
AF = mybir.ActivationFunctionType
NPBF16 = ml_dtypes.bfloat16

B, S, D, H, DH = 2, 2048, 768, 12, 64
N_CORES = 8
DCH = D // 128          # 6 d_model chunks
NKT = S // 128          # 16 k tiles
QB = 512                # q block width
NQB = S // QB           # 4 q blocks
G = 2                   # k-tiles per exp group

TRACE_ENABLED = False
LAST_EXEC_NS = None
LAST_RESULT = None
_BUILT = None


def build_nc():
    nc = bacc.Bacc("TRN2", target_bir_lowering=False, debug=False)

    xT_d = nc.dram_tensor("xT", [D, S], BF16, kind="ExternalInput")
    wq01_d = nc.dram_tensor("wq01", [D, 128], BF16, kind="ExternalInput")
    wk01_d = nc.dram_tensor("wk01", [D, 128], BF16, kind="ExternalInput")
    wqk2_d = nc.dram_tensor("wqk2", [D, 128], BF16, kind="ExternalInput")
    wv_d = nc.dram_tensor("wv", [D, 195], BF16, kind="ExternalInput")
    wo01_d = nc.dram_tensor("wo01", [128, D], BF16, kind="ExternalInput")
    wo2_d = nc.dram_tensor("wo2", [64, D], BF16, kind="ExternalInput")
    bq01_d = nc.dram_tensor("bq01", [128, 1], F32, kind="ExternalInput")
    bk01_d = nc.dram_tensor("bk01", [128, 1], F32, kind="ExternalInput")
    bqk2_d = nc.dram_tensor("bqk2", [128, 1], F32, kind="ExternalInput")
    bv_d = nc.dram_tensor("bv", [128, 195], F32, kind="ExternalInput")
    out_d = nc.dram_tensor("out_p", [S, D], F32, kind="ExternalOutput")

    tri_np = np.triu(np.ones((128, 128), np.float32)).astype(NPBF16)
    tri_d = nc.inline_tensor(tri_np, "tri")
    ones_d = nc.inline_tensor(np.ones((128, 1), NPBF16), "ones_w")

    with tile.TileContext(nc) as tc, ExitStack() as ctx:
        persist = ctx.enter_context(tc.tile_pool(name="persist", bufs=1))

        # ---- stage inputs in SBUF ----
        xt = []
        for d in range(DCH):
            t = persist.tile([128, S], BF16, tag=f"xt{d}")
            nc.sync.dma_start(t[:], xT_d[d * 128:(d + 1) * 128, :])
            xt.append(t)

        def load_w(dram, cols, tag):
            t = persist.tile([128, DCH * cols], BF16, tag=tag)
            for d in range(DCH):
                nc.sync.dma_start(t[:, d * cols:(d + 1) * cols],
                                  dram[d * 128:(d + 1) * 128, :])
            return t

        wq01 = load_w(wq01_d, 128, "wq01")
        wk01 = load_w(wk01_d, 128, "wk01")
        wqk2 = load_w(wqk2_d, 128, "wqk2")
        wv = load_w(wv_d, 195, "wv")

        wo01 = persist.tile([128, D], BF16, tag="wo01")
        nc.sync.dma_start(wo01[:], wo01_d[:, :])
        wo2 = persist.tile([64, D], BF16, tag="wo2")
        nc.sync.dma_start(wo2[:], wo2_d[:, :])

        def load_small(dram, shape, dt, tag):
            t = persist.tile(shape, dt, tag=tag)
            nc.sync.dma_start(t[:], dram[:, :])
            return t

        bq01 = load_small(bq01_d, [128, 1], F32, "bq01")
        bk01 = load_small(bk01_d, [128, 1], F32, "bk01")
        bqk2 = load_small(bqk2_d, [128, 1], F32, "bqk2")
        bv = load_small(bv_d, [128, 195], F32, "bv")
        tri = load_small(tri_d, [128, 128], BF16, "tri")
        ones_w = load_small(ones_d, [128, 1], BF16, "ones_w")

        # ---- persistent intermediates ----
        QT01 = persist.tile([128, S], BF16, tag="QT01")
        KT01 = persist.tile([128, S], BF16, tag="KT01")
        QKT2 = persist.tile([128, S], BF16, tag="QKT2")
        KT2lo = persist.tile([64, S], BF16, tag="KT2lo")
        # V augmented with a ones column per head ([V_h | 1] x 3, 195 cols per
        # s-tile) so the PV matmul's 65th output row is the softmax denominator
        v_sb = persist.tile([128, NKT * 195], BF16, tag="v_sb")
        zT01 = persist.tile([128, S], BF16, tag="zT01")
        zT2 = persist.tile([64, S], BF16, tag="zT2")

        # ---- QKV projections ----
        with tc.tile_pool(name="proj_ps", bufs=1, space="PSUM") as proj_pool, \
             tc.tile_pool(name="v_ps", bufs=2, space="PSUM") as v_pool:
            for w_s, bias_s, out_s in ((wq01, bq01, QT01),
                                       (wk01, bk01, KT01),
                                       (wqk2, bqk2, QKT2)):
                ps = proj_pool.tile([128, S], F32, tag="chain")
                for d in range(DCH):
                    lhsT = w_s[:, d * 128:(d + 1) * 128]
                    for n in range(S // 512):
                        nc.tensor.matmul(ps[:, n * 512:(n + 1) * 512], lhsT,
                                         xt[d][:, n * 512:(n + 1) * 512],
                                         start=(d == 0), stop=(d == DCH - 1))
                for n in range(S // 512):
                    nsl = slice(n * 512, (n + 1) * 512)
                    nc.vector.tensor_scalar_add(out_s[:, nsl], ps[:, nsl],
                                                bias_s[:])

            for s_t in range(NKT):
                ps = v_pool.tile([128, 195], F32, tag="vps")
                for d in range(DCH):
                    nc.tensor.matmul(ps[:], xt[d][:, s_t * 128:(s_t + 1) * 128],
                                     wv[:, d * 195:(d + 1) * 195],
                                     start=(d == 0), stop=(d == DCH - 1))
                nc.vector.tensor_add(v_sb[:, s_t * 195:(s_t + 1) * 195], ps[:], bv[:])

        # head-2 K^T copy to partitions 0-63 (scores h2 needs lhsT and rhs on
        # the same partition range)
        nc.sync.dma_start(KT2lo[:], QKT2[64:128, :])

        # ---- attention ----
        with tc.tile_pool(name="sT_ps", bufs=2, space="PSUM") as sT_pool, \
             tc.tile_pool(name="zT_ps", bufs=4, space="PSUM") as zT_pool, \
             tc.tile_pool(name="pt_sb", bufs=4) as pt_pool, \
             tc.tile_pool(name="rb_sb", bufs=3) as rb_pool, \
             tc.tile_pool(name="zs_sb", bufs=2) as zs_pool, \
             tc.tile_pool(name="recip_dr", bufs=2, space="DRAM") as rdr_pool, \
             tc.tile_pool(name="recip_sb", bufs=2) as recip_pool:

            for qi in range(NQB):
                q0 = qi * QB
                J = 4 * qi + 4
                qsl = slice(q0, q0 + QB)

                def head_loop(score_lhs, score_rhs, po, hv, zt_out):
                    """Emit scores+exp+mask+PV for one head's (qi) row.

                    po: partition offset (0 or 64) of this head's K^T/Q^T rows
                    inside score_lhs/score_rhs — also selects the PE row strip.
                    hv: head index (0-2) into the augmented V tile; the PV
                    matmul's lhsT is [V_h | 1] so zt_out row 64 accumulates
                    the softmax denominator.
                    """
                    for g in range(J // G):
                        st = sT_pool.tile([128, G * 512], F32, tag="sT")
                        for jj in range(G):
                            j = g * G + jj
                            nc.tensor.matmul(
                                st[:, jj * 512:(jj + 1) * 512],
                                score_lhs[po:po + 64, j * 128:(j + 1) * 128],
                                score_rhs[po:po + 64, qsl],
                                start=True, stop=True)
                        pt = pt_pool.tile([128, G * 512], BF16, tag="pt")
                        j0 = g * G
                        r0 = j0 - 4 * qi
                        s0 = r0 * 128 if r0 >= 0 else 0
                        nc.scalar.activation(pt[:, s0:G * 512], st[:, s0:G * 512],
                                             AF.Exp)
                        for jj in range(G):
                            j = g * G + jj
                            r = j - 4 * qi
                            off = jj * 512
                            if r >= 0:
                                if r > 0:
                                    nc.vector.memset(pt[:, off:off + r * 128], 0.0)
                                dsl = slice(off + r * 128, off + (r + 1) * 128)
                                nc.vector.tensor_mul(pt[:, dsl], pt[:, dsl], tri[:])
                        for jj in range(G):
                            j = g * G + jj
                            nc.tensor.matmul(
                                zt_out[:],
                                v_sb[:, j * 195 + hv * 65:j * 195 + hv * 65 + 65],
                                pt[:, jj * 512:(jj + 1) * 512],
                                start=(j == 0), stop=(j == J - 1))

                zts = [zT_pool.tile([65, 512], F32, tag="zT", name=f"zt{i}")
                       for i in range(3)]
                head_loop(KT01, QT01, 0, 0, zts[0])
                head_loop(KT01, QT01, 64, 1, zts[1])
                head_loop(KT2lo, QKT2, 0, 2, zts[2])

                # normalize: zT_h = zt_h[0:64] * broadcast(1 / zt_h[64]).
                # The reciprocal of the 3x512 sums is done on a [128, 12]
                # reshape (via a DRAM bounce) — InstReciprocal costs ~6.5ns
                # per FREE element, so the [1, 512] layout would be 3.3us.
                s3 = recip_pool.tile([1, 3 * 512], F32, tag="s3")
                for h in range(3):
                    nc.vector.tensor_copy(s3[:, h * 512:(h + 1) * 512],
                                          zts[h][64:65, :])
                dr1 = rdr_pool.tile([1, 3 * 512], F32, tag="dr1")
                nc.sync.dma_start(dr1[:], s3[:])
                rs = recip_pool.tile([128, 12], F32, tag="rs")
                nc.sync.dma_start(
                    rs[:], dr1[:].rearrange("o (p f) -> (o p) f", p=128))
                rr = recip_pool.tile([128, 12], F32, tag="rr")
                nc.vector.reciprocal(rr[:], rs[:])
                dr2 = rdr_pool.tile([1, 3 * 512], F32, tag="dr2")
                nc.sync.dma_start(
                    dr2[:].rearrange("o (p f) -> (o p) f", p=128), rr[:])
                rb = rb_pool.tile([64, 3 * 512], F32, tag="rb")
                for h in range(3):
                    nc.sync.dma_start(
                        rb[:, h * 512:(h + 1) * 512],
                        dr2[0:1, h * 512:(h + 1) * 512].broadcast_to([64, 512]))

                nc.vector.tensor_mul(zT01[0:64, qsl], zts[0][0:64, :],
                                     rb[:, 0:512])
                z1 = zs_pool.tile([64, 512], BF16, tag="z1")
                nc.vector.tensor_mul(z1[:], zts[1][0:64, :], rb[:, 512:1024])
                # head 1 lives on partitions 64-127 of zT01: DMA partition-shift
                nc.sync.dma_start(zT01[64:128, qsl], z1[:])
                nc.vector.tensor_mul(zT2[:, qsl], zts[2][0:64, :],
                                     rb[:, 1024:1536])

        # ---- output projection ----
        with tc.tile_pool(name="op_ps", bufs=2, space="PSUM") as op_pool, \
             tc.tile_pool(name="out_sb", bufs=3) as out_pool:
            for t in range(NKT):
                ps = op_pool.tile([128, D], F32, tag="op")
                tsl = slice(t * 128, (t + 1) * 128)
                for n0, nw in ((0, 512), (512, 256)):
                    nc.tensor.matmul(ps[:, n0:n0 + nw], zT01[:, tsl],
                                     wo01[:, n0:n0 + nw], start=True, stop=False)
                for n0, nw in ((0, 512), (512, 256)):
                    nc.tensor.matmul(ps[:, n0:n0 + nw], zT2[:, tsl],
                                     wo2[:, n0:n0 + nw], start=False, stop=True)
                ob = out_pool.tile([128, D], F32, tag="ob")
                nc.vector.tensor_copy(ob[:], ps[:])
                nc.sync.dma_start(out_d[tsl, :], ob[:])

    nc.compile()
    return nc


def _get_nc():
    global _BUILT
    if _BUILT is None:
        _BUILT = build_nc()
    return _BUILT


def make_in_maps(inputs):
    x = np.asarray(inputs["normalized_resid_pre"], dtype=np.float32)
    W_Q = np.asarray(inputs["W_Q"], dtype=np.float32)
    W_K = np.asarray(inputs["W_K"], dtype=np.float32)
    W_V = np.asarray(inputs["W_V"], dtype=np.float32)
    W_O = np.asarray(inputs["W_O"], dtype=np.float32)
    b_Q = np.asarray(inputs["b_Q"], dtype=np.float32)
    b_K = np.asarray(inputs["b_K"], dtype=np.float32)
    b_V = np.asarray(inputs["b_V"], dtype=np.float32)
    sc = 1.0 / np.sqrt(np.float32(DH))

    in_maps = []
    for c in range(N_CORES):
        b = c // 4
        h = (c % 4) * 3
        hs = [h, h + 1, h + 2]
        m = {
            "xT": np.ascontiguousarray(x[b].T).astype(NPBF16),
            "wq01": np.concatenate([W_Q[hs[0]] * sc, W_Q[hs[1]] * sc],
                                   axis=1).astype(NPBF16),
            "wk01": np.concatenate([W_K[hs[0]], W_K[hs[1]]], axis=1).astype(NPBF16),
            "wqk2": np.concatenate([W_Q[hs[2]] * sc, W_K[hs[2]]],
                                   axis=1).astype(NPBF16),
            "wv": np.concatenate(
                sum(([W_V[hh], np.zeros((D, 1), np.float32)] for hh in hs), []),
                axis=1).astype(NPBF16),
            "wo01": np.concatenate([W_O[hs[0]], W_O[hs[1]]], axis=0).astype(NPBF16),
            "wo2": W_O[hs[2]].astype(NPBF16),
            "bq01": (np.concatenate([b_Q[hs[0]], b_Q[hs[1]]]) * sc)[:, None]
                    .astype(np.float32),
            "bk01": np.concatenate([b_K[hs[0]], b_K[hs[1]]])[:, None]
                    .astype(np.float32),
            "bqk2": np.concatenate([b_Q[hs[2]] * sc, b_K[hs[2]]])[:, None]
                    .astype(np.float32),
            "bv": np.ascontiguousarray(np.broadcast_to(
                np.concatenate(
                    sum(([b_V[hh], np.ones(1, np.float32)] for hh in hs), [])),
                (128, 195))).astype(np.float32),
        }
        in_maps.append(m)
    return in_maps


def kernel(**inputs):
    global LAST_EXEC_NS, LAST_RESULT
    nc = _get_nc()
    in_maps = make_in_maps(inputs)
    b_O = np.asarray(inputs["b_O"], dtype=np.float32)

    res = run_bass_kernel_spmd(nc, in_maps, core_ids=list(range(N_CORES)),
                               trace=TRACE_ENABLED)
    LAST_EXEC_NS = res.exec_time_ns
    LAST_RESULT = res
    parts = [r["out_p"] for r in res.results]
    out0 = parts[0] + parts[1] + parts[2] + parts[3]
    out1 = parts[4] + parts[5] + parts[6] + parts[7]
    out = np.stack([out0, out1]) + b_O
    return out.astype(np.float32)


# revision 41
# speedup vs baseline: 1.2394x; 1.0599x over previous
"""Multi-head causal attention (dense transformer block) on 8 TRN2 NeuronCores.

Problem: B=2, S=2048, D_MODEL=768, H=12 heads, D_HEAD=64, fp32 I/O.

Sharding: 24 (batch, head) units over 8 cores -> 3 heads x 1 batch per core.
Cores 0-3 handle batch 0 (heads 0-2, 3-5, 6-8, 9-11), cores 4-7 batch 1.
Each core computes its heads' contribution to out[b] = sum_h z_h @ W_O[h];
the host sums the 4 partials per batch and adds b_O (an all-reduce done on
the host since the kernel contract is full-in/full-out).

Per-core dataflow (all matmuls bf16 -> fp32 PSUM):
  - x[b]^T staged in SBUF as 6 chunks [128, 2048].
  - QT/KT in "head-transposed" layout [64, S] packed 2-per-tile:
      QT01 = [Q_h0*s | Q_h1*s]^T x  (scale 1/sqrt(64) folded into W_Q/b_Q)
      KT01 = [K_h0 | K_h1]^T x, QKT2 = [Q_h2*s | K_h2]^T x
  - scores TRANSPOSED: sT[k, q] = KT^T-slice x QT (K=64 contraction), so
    softmax normalization is a per-q (free dim) reciprocal and no PE
    transposes of the probability matrix are ever needed.
  - exp via ScalarE (one instr per [128, 1024] j-pair group), causal mask via
    memset + one triangular-mask multiply on the diagonal 128x128 tile.
  - zT[h] = sum_j V[j]^T-slice x PT[j]  (unnormalized), denominators via a
    ones-vector matmul (M=1) accumulated alongside, applied as
    zT * broadcast(1/sums) when copying PSUM->SBUF.
  - out[q, :] = zT01^T-slice @ WO01 + zT2^T-slice @ WO2, DMA'd PSUM->DRAM.
"""

import numpy as np
import ml_dtypes
from contextlib import ExitStack

import concourse.bass as bass
import concourse.mybir as mybir
import concourse.tile as tile
from concourse import bacc
from concourse.bass_utils import run_bass_kernel_spmd

BF16 = mybir.dt.bfloat16
F32 = mybir.dt.float32
AF = mybir.ActivationFunctionType
NPBF16 = ml_dtypes.bfloat16

B, S, D, H, DH = 2, 2048, 768, 12, 64
N_CORES = 8
DCH = D // 128          # 6 d_model chunks
NKT = S // 128          # 16 k tiles
QB = 512                # q block width
NQB = S // QB           # 4 q blocks
G = 2                   # k-tiles per exp group

TRACE_ENABLED = False
LAST_EXEC_NS = None
LAST_RESULT = None
_BUILT = None


def build_nc():
    nc = bacc.Bacc("TRN2", target_bir_lowering=False, debug=False)

    xT_d = nc.dram_tensor("xT", [D, S], BF16, kind="ExternalInput")
    wq01_d = nc.dram_tensor("wq01", [D, 128], BF16, kind="ExternalInput")
    wk01_d = nc.dram_tensor("wk01", [D, 128], BF16, kind="ExternalInput")
    wqk2_d = nc.dram_tensor("wqk2", [D, 128], BF16, kind="ExternalInput")
    wv_d = nc.dram_tensor("wv", [D, 195], BF16, kind="ExternalInput")
    wo01_d = nc.dram_tensor("wo01", [128, D], BF16, kind="ExternalInput")
    wo2_d = nc.dram_tensor("wo2", [64, D], BF16, kind="ExternalInput")
    bq01_d = nc.dram_tensor("bq01", [128, 1], F32, kind="ExternalInput")
    bk01_d = nc.dram_tensor("bk01", [128, 1], F32, kind="ExternalInput")
    bqk2_d = nc.dram_tensor("bqk2", [128, 1], F32, kind="ExternalInput")
    bv_d = nc.dram_tensor("bv", [128, 195], F32, kind="ExternalInput")
    out_d = nc.dram_tensor("out_p", [S, D], F32, kind="ExternalOutput")

    tri_np = np.triu(np.ones((128, 128), np.float32)).astype(NPBF16)
    tri_d = nc.inline_tensor(tri_np, "tri")
    ones_d = nc.inline_tensor(np.ones((128, 1), NPBF16), "ones_w")

    with tile.TileContext(nc) as tc, ExitStack() as ctx:
        persist = ctx.enter_context(tc.tile_pool(name="persist", bufs=1))

        # ---- stage inputs in SBUF ----
        xt = []
        for d in range(DCH):
            t = persist.tile([128, S], BF16, tag=f"xt{d}")
            nc.sync.dma_start(t[:], xT_d[d * 128:(d + 1) * 128, :])
            xt.append(t)

        def load_w(dram, cols, tag):
            # one DMA: [D, cols] DRAM -> [128, DCH*cols] SBUF (d-chunks along
            # the free dim)
            t = persist.tile([128, DCH * cols], BF16, tag=tag)
            a = dram[:, :]
            src = bass.AP(tensor=a.tensor, offset=a.offset,
                          ap=[[cols, 128], [128 * cols, DCH], [1, cols]])
            nc.sync.dma_start(t[:].rearrange("p (c f) -> p c f", c=DCH), src)
            return t

        wq01 = load_w(wq01_d, 128, "wq01")
        wk01 = load_w(wk01_d, 128, "wk01")
        wqk2 = load_w(wqk2_d, 128, "wqk2")
        wv = load_w(wv_d, 195, "wv")

        wo01 = persist.tile([128, D], BF16, tag="wo01")
        nc.sync.dma_start(wo01[:], wo01_d[:, :])
        wo2 = persist.tile([64, D], BF16, tag="wo2")
        nc.sync.dma_start(wo2[:], wo2_d[:, :])

        def load_small(dram, shape, dt, tag):
            t = persist.tile(shape, dt, tag=tag)
            nc.sync.dma_start(t[:], dram[:, :])
            return t

        bq01 = load_small(bq01_d, [128, 1], F32, "bq01")
        bk01 = load_small(bk01_d, [128, 1], F32, "bk01")
        bqk2 = load_small(bqk2_d, [128, 1], F32, "bqk2")
        bv = load_small(bv_d, [128, 195], F32, "bv")
        tri = load_small(tri_d, [128, 128], BF16, "tri")
        ones_w = load_small(ones_d, [128, 1], BF16, "ones_w")

        # ---- persistent intermediates ----
        QT01 = persist.tile([128, S], BF16, tag="QT01")
        KT01 = persist.tile([128, S], BF16, tag="KT01")
        QKT2 = persist.tile([128, S], BF16, tag="QKT2")
        KT2lo = persist.tile([64, S], BF16, tag="KT2lo")
        # V augmented with a ones column per head ([V_h | 1] x 3, 195 cols per
        # s-tile) so the PV matmul's 65th output row is the softmax denominator
        v_sb = persist.tile([128, NKT * 195], BF16, tag="v_sb")
        zT01 = persist.tile([128, S], BF16, tag="zT01")
        zT2 = persist.tile([64, S], BF16, tag="zT2")

        # ---- QKV projections ----
        with tc.tile_pool(name="proj_ps", bufs=1, space="PSUM") as proj_pool, \
             tc.tile_pool(name="v_ps", bufs=2, space="PSUM") as v_pool:
            for w_s, bias_s, out_s in ((wq01, bq01, QT01),
                                       (wk01, bk01, KT01),
                                       (wqk2, bqk2, QKT2)):
                ps = proj_pool.tile([128, S], F32, tag="chain")
                for d in range(DCH):
                    lhsT = w_s[:, d * 128:(d + 1) * 128]
                    for n in range(S // 512):
                        nc.tensor.matmul(ps[:, n * 512:(n + 1) * 512], lhsT,
                                         xt[d][:, n * 512:(n + 1) * 512],
                                         start=(d == 0), stop=(d == DCH - 1))
                for n in range(S // 512):
                    nsl = slice(n * 512, (n + 1) * 512)
                    nc.vector.tensor_scalar_add(out_s[:, nsl], ps[:, nsl],
                                                bias_s[:])

            for s_t in range(NKT):
                ps = v_pool.tile([128, 195], F32, tag="vps")
                for d in range(DCH):
                    nc.tensor.matmul(ps[:], xt[d][:, s_t * 128:(s_t + 1) * 128],
                                     wv[:, d * 195:(d + 1) * 195],
                                     start=(d == 0), stop=(d == DCH - 1))
                nc.vector.tensor_add(v_sb[:, s_t * 195:(s_t + 1) * 195], ps[:], bv[:])

        # head-2 K^T copy to partitions 0-63 (scores h2 needs lhsT and rhs on
        # the same partition range)
        nc.sync.dma_start(KT2lo[:], QKT2[64:128, :])

        # ---- attention ----
        with tc.tile_pool(name="sT_ps", bufs=2, space="PSUM") as sT_pool, \
             tc.tile_pool(name="zT_ps", bufs=4, space="PSUM") as zT_pool, \
             tc.tile_pool(name="pt_sb", bufs=6) as pt_pool, \
             tc.tile_pool(name="rb_sb", bufs=3) as rb_pool, \
             tc.tile_pool(name="zs_sb", bufs=2) as zs_pool, \
             tc.tile_pool(name="recip_dr", bufs=2, space="DRAM") as rdr_pool, \
             tc.tile_pool(name="recip_sb", bufs=2) as recip_pool:

            for qi in range(NQB):
                q0 = qi * QB
                J = 4 * qi + 4
                qsl = slice(q0, q0 + QB)

                def exp_mask(g, st, name):
                    pt = pt_pool.tile([128, G * 512], BF16, tag="pt", name=name)
                    r0 = g * G - 4 * qi
                    s0 = r0 * 128 if r0 >= 0 else 0
                    nc.scalar.activation(pt[:, s0:G * 512], st[:, s0:G * 512],
                                         AF.Exp)
                    for jj in range(G):
                        r = (g * G + jj) - 4 * qi
                        off = jj * 512
                        if r >= 0:
                            if r > 0:
                                nc.vector.memset(pt[:, off:off + r * 128], 0.0)
                            dsl = slice(off + r * 128, off + (r + 1) * 128)
                            nc.vector.tensor_mul(pt[:, dsl], pt[:, dsl], tri[:])
                    return pt

                def pv(g, pt, hv, zt_out):
                    for jj in range(G):
                        j = g * G + jj
                        nc.tensor.matmul(
                            zt_out[:],
                            v_sb[:, j * 195 + hv * 65:j * 195 + hv * 65 + 65],
                            pt[:, jj * 512:(jj + 1) * 512],
                            start=(j == 0), stop=(j == J - 1))

                zts = [zT_pool.tile([65, 512], F32, tag="zT", name=f"zt{i}")
                       for i in range(3)]
                # heads 0+1 interleaved: the two score matmuls of a (g, jj)
                # target different PE row strips (0-63 / 64-127) and can
                # overlap in the array when issued back-to-back
                for g in range(J // G):
                    st0 = sT_pool.tile([128, G * 512], F32, tag="sT", name="st0")
                    st1 = sT_pool.tile([128, G * 512], F32, tag="sT", name="st1")
                    for jj in range(G):
                        j = g * G + jj
                        ksl = slice(j * 128, (j + 1) * 128)
                        osl = slice(jj * 512, (jj + 1) * 512)
                        nc.tensor.matmul(st0[:, osl], KT01[0:64, ksl],
                                         QT01[0:64, qsl], start=True, stop=True)
                        nc.tensor.matmul(st1[:, osl], KT01[64:128, ksl],
                                         QT01[64:128, qsl], start=True, stop=True)
                    pt0 = exp_mask(g, st0, "pt0")
                    pt1 = exp_mask(g, st1, "pt1")
                    pv(g, pt0, 0, zts[0])
                    pv(g, pt1, 1, zts[1])
                # head 2
                for g in range(J // G):
                    st2 = sT_pool.tile([128, G * 512], F32, tag="sT", name="st2")
                    for jj in range(G):
                        j = g * G + jj
                        nc.tensor.matmul(
                            st2[:, jj * 512:(jj + 1) * 512],
                            KT2lo[0:64, j * 128:(j + 1) * 128],
                            QKT2[0:64, qsl], start=True, stop=True)
                    pt2 = exp_mask(g, st2, "pt2")
                    pv(g, pt2, 2, zts[2])

                # normalize: zT_h = zt_h[0:64] * broadcast(1 / zt_h[64]).
                # The reciprocal of the 3x512 sums is done on a [128, 12]
                # reshape (via a DRAM bounce) — InstReciprocal costs ~6.5ns
                # per FREE element, so the [1, 512] layout would be 3.3us.
                s3 = recip_pool.tile([1, 3 * 512], F32, tag="s3")
                for h in range(3):
                    nc.vector.tensor_copy(s3[:, h * 512:(h + 1) * 512],
                                          zts[h][64:65, :])
                dr1 = rdr_pool.tile([1, 3 * 512], F32, tag="dr1")
                nc.sync.dma_start(dr1[:], s3[:])
                rs = recip_pool.tile([128, 12], F32, tag="rs")
                nc.sync.dma_start(
                    rs[:], dr1[:].rearrange("o (p f) -> (o p) f", p=128))
                rr = recip_pool.tile([128, 12], F32, tag="rr")
                nc.vector.reciprocal(rr[:], rs[:])
                dr2 = rdr_pool.tile([1, 3 * 512], F32, tag="dr2")
                nc.sync.dma_start(
                    dr2[:].rearrange("o (p f) -> (o p) f", p=128), rr[:])
                rb = rb_pool.tile([64, 3 * 512], F32, tag="rb")
                for h in range(3):
                    nc.sync.dma_start(
                        rb[:, h * 512:(h + 1) * 512],
                        dr2[0:1, h * 512:(h + 1) * 512].broadcast_to([64, 512]))

                nc.vector.tensor_mul(zT01[0:64, qsl], zts[0][0:64, :],
                                     rb[:, 0:512])
                z1 = zs_pool.tile([64, 512], BF16, tag="z1")
                nc.vector.tensor_mul(z1[:], zts[1][0:64, :], rb[:, 512:1024])
                # head 1 lives on partitions 64-127 of zT01: DMA partition-shift
                nc.sync.dma_start(zT01[64:128, qsl], z1[:])
                nc.vector.tensor_mul(zT2[:, qsl], zts[2][0:64, :],
                                     rb[:, 1024:1536])

        # ---- output projection ----
        with tc.tile_pool(name="op_ps", bufs=2, space="PSUM") as op_pool, \
             tc.tile_pool(name="out_sb", bufs=3) as out_pool:
            for t in range(NKT):
                ps = op_pool.tile([128, D], F32, tag="op")
                tsl = slice(t * 128, (t + 1) * 128)
                for n0, nw in ((0, 512), (512, 256)):
                    nc.tensor.matmul(ps[:, n0:n0 + nw], zT01[:, tsl],
                                     wo01[:, n0:n0 + nw], start=True, stop=False)
                for n0, nw in ((0, 512), (512, 256)):
                    nc.tensor.matmul(ps[:, n0:n0 + nw], zT2[:, tsl],
                                     wo2[:, n0:n0 + nw], start=False, stop=True)
                ob = out_pool.tile([128, D], F32, tag="ob")
                nc.vector.tensor_copy(ob[:], ps[:])
                nc.sync.dma_start(out_d[tsl, :], ob[:])

    nc.compile()
    return nc


def _get_nc():
    global _BUILT
    if _BUILT is None:
        _BUILT = build_nc()
    return _BUILT


def make_in_maps(inputs):
    x = np.asarray(inputs["normalized_resid_pre"], dtype=np.float32)
    W_Q = np.asarray(inputs["W_Q"], dtype=np.float32)
    W_K = np.asarray(inputs["W_K"], dtype=np.float32)
    W_V = np.asarray(inputs["W_V"], dtype=np.float32)
    W_O = np.asarray(inputs["W_O"], dtype=np.float32)
    b_Q = np.asarray(inputs["b_Q"], dtype=np.float32)
    b_K = np.asarray(inputs["b_K"], dtype=np.float32)
    b_V = np.asarray(inputs["b_V"], dtype=np.float32)
    sc = 1.0 / np.sqrt(np.float32(DH))

    in_maps = []
    for c in range(N_CORES):
        b = c // 4
        h = (c % 4) * 3
        hs = [h, h + 1, h + 2]
        m = {
            "xT": np.ascontiguousarray(x[b].T).astype(NPBF16),
            "wq01": np.concatenate([W_Q[hs[0]] * sc, W_Q[hs[1]] * sc],
                                   axis=1).astype(NPBF16),
            "wk01": np.concatenate([W_K[hs[0]], W_K[hs[1]]], axis=1).astype(NPBF16),
            "wqk2": np.concatenate([W_Q[hs[2]] * sc, W_K[hs[2]]],
                                   axis=1).astype(NPBF16),
            "wv": np.concatenate(
                sum(([W_V[hh], np.zeros((D, 1), np.float32)] for hh in hs), []),
                axis=1).astype(NPBF16),
            "wo01": np.concatenate([W_O[hs[0]], W_O[hs[1]]], axis=0).astype(NPBF16),
            "wo2": W_O[hs[2]].astype(NPBF16),
            "bq01": (np.concatenate([b_Q[hs[0]], b_Q[hs[1]]]) * sc)[:, None]
                    .astype(np.float32),
            "bk01": np.concatenate([b_K[hs[0]], b_K[hs[1]]])[:, None]
                    .astype(np.float32),
            "bqk2": np.concatenate([b_Q[hs[2]] * sc, b_K[hs[2]]])[:, None]
                    .astype(np.float32),
            "bv": np.ascontiguousarray(np.broadcast_to(
                np.concatenate(
                    sum(([b_V[hh], np.ones(1, np.float32)] for hh in hs), [])),
                (128, 195))).astype(np.float32),
        }
        in_maps.append(m)
    return in_maps


def kernel(**inputs):
    global LAST_EXEC_NS, LAST_RESULT
    nc = _get_nc()
    in_maps = make_in_maps(inputs)
    b_O = np.asarray(inputs["b_O"], dtype=np.float32)

    res = run_bass_kernel_spmd(nc, in_maps, core_ids=list(range(N_CORES)),
                               trace=TRACE_ENABLED)
    LAST_EXEC_NS = res.exec_time_ns
    LAST_RESULT = res
    parts = [r["out_p"] for r in res.results]
    out0 = parts[0] + parts[1] + parts[2] + parts[3]
    out1 = parts[4] + parts[5] + parts[6] + parts[7]
    out = np.stack([out0, out1]) + b_O
    return out.astype(np.float32)
